# revision 67
# baseline (speedup 1.0000x reference)
"""CombinedRotaryEmbedding Trainium2 kernel.

Math (per 64-dim head, per position s, with R = composed Givens @ rotation_matrix):
    u = x @ R[:, 0::2],  v = x @ R[:, 1::2]
    out = [u*cos - v*sin | u*sin + v*cos]     cos/sin = f(position, freq[32])

Karatsuba-style 3-product form: per head the matmul produces
    z1 = x @ (R_even+R_odd),  z2 = x @ R_even,  z3 = x @ R_odd
and the vector engine computes k = [z1*cos | z2*(sin-cos) | z3*(cos+sin)],
then out_lo = k1 - k3 and out_hi = k1 + k2: 1.5 multiplies per output
element (vs 2 for the direct form), keeping the DVE stream under the DMA
floor. The [z1|z2|z3] blocks for 2 heads (192 wide) extend to a 256-wide
float32r matmul (1 cycle/row); the 64 "pad" columns physically overlap the
identity block in SBUF, since their PSUM output is never read.

Kernel strategy (8-way data parallel over the sequence dim):
  - host: one [128, 576] constant block: [RR(192) | identity(128) |
    cos rows(128) | sin rows(128)]; gpsimd builds the [c|s-c|c+s]
    coefficient tile on-chip.
  - device, per core (x shard [2048 rows, 1024] = 16 subtiles of 128 rows):
      SP   : all 16 subtile loads upfront (subtiles 0-2 split for pipeline
             fill), then one full-subtile store each; loads first means a
             store's sem wait can never delay a load
      PE   : transpose x chunks via f32r identity, one 256-wide f32r matmul
             per 2-head chunk -> [z1|z2|z3] blocks in PSUM
      ACT  : copy transposed chunks PSUM -> SBUF (4-chunk groups)
      DVE  : k = [z1|z2|z3] * coeffs (PSUM -> SBUF, [3,3,2]-chunk groups)
             plus one of the four add-ops per subtile
      GPSIMD: out_lo/out_hi adds per half-subtile
    The DMA engine is the binding resource and runs with zero idle: total =
    DMA start (1966) + 47.4 us busy + store sem (900) + drain (744).
    Compute finishes ~3 us before the last store.
"""

import numpy as np

import concourse.bacc as bacc
import concourse.bass as bass
import concourse.tile as tile
from concourse import mybir
from concourse.bass_utils import run_bass_kernel_spmd

N_CORES = 8
B, S, N_STATE = 4, 4096, 1024
H, D = 16, 64            # heads, head dim
HALF = D // 2            # 32 rotary freqs
S_SH = S // N_CORES      # 512 positions per core
ROWS = B * S_SH          # 2048 rows of [1024] per core
NBLK = ROWS // 256       # 8 DMA blocks of 256 rows
CBLK = S_SH // 128       # 4 distinct position blocks per core
NCH = N_STATE // 128     # 8 two-head chunks per row
F32 = mybir.dt.float32
F32R = mybir.dt.float32r
BF16 = mybir.dt.bfloat16

_compiled = {}


def _build_nc():
    nc = bacc.Bacc("TRN2")
    x_in = nc.dram_tensor("x", [ROWS, N_STATE], F32R, kind="ExternalInput")
    # combined constant block [RR(192) | identity(128) | cos | sin rows]:
    # RR holds per-head [R_even+R_odd | R_even | R_odd] blocks; its 256-wide
    # f32r matmul view extends 64 cols into the identity (that PSUM output
    # range is never read)
    consts_in = nc.dram_tensor("consts", [128, 576], F32R,
                               kind="ExternalInput")
    out_d = nc.dram_tensor("out", [ROWS, N_STATE], F32, kind="ExternalOutput")

    with tile.TileContext(nc) as tc:
        with (
            tc.tile_pool(name="const", bufs=1) as const,
            tc.tile_pool(name="xin", bufs=2 * NBLK) as xin,
            tc.tile_pool(name="xtp", bufs=3) as xtp,
            tc.tile_pool(name="tpsum", bufs=2, space="PSUM") as tpsum,
            tc.tile_pool(name="ypsum", bufs=2, space="PSUM") as ypsum,
            tc.tile_pool(name="tsb", bufs=3) as tsb,
            # one buffer per subtile: output buffers must never gate compute
            # (stores drain late because loads occupy the DMA engines first)
            tc.tile_pool(name="outp", bufs=2 * NBLK) as outp,
        ):
            # first subtile load goes ahead of everything else, split in two
            # so the transpose->copy->matmul->mul chain starts as early as
            # possible (chunks 0-3 usable after a 256 KB transfer)
            xts = []
            x_t = xin.tile([128, N_STATE], F32R, tag="xin", name="xs0")
            nc.sync.dma_start(out=x_t[:, 0:512], in_=x_in[0:128, 0:512])
            xts.append(x_t)

            consts = const.tile([128, 576], F32R)
            nc.sync.dma_start(out=consts[:], in_=consts_in[:])
            nc.sync.dma_start(out=x_t[:, 512:1024], in_=x_in[0:128, 512:1024])
            # build [cos | sin-cos | cos+sin] per position block from the
            # compact [cos | sin] rows (gpsimd is idle this early); DMAing
            # the compact form saves 182 ns on the binding DMA stream
            coef = const.tile([128, CBLK, 96], F32)
            cvb = consts[:].bitcast(F32)
            kb = coef[:]

            def _cs(off):
                # c rows at 320 + cb*32, s rows at 448 + cb*32
                return bass.AP(tensor=cvb.tensor, offset=cvb.offset + off,
                               ap=[list(cvb.ap[0]), [32, CBLK], [1, 32]])

            def _ko(dout):
                return bass.AP(tensor=kb.tensor, offset=kb.offset + dout,
                               ap=[list(kb.ap[0]), [96, CBLK], [1, 32]])

            nc.gpsimd.tensor_copy(_ko(0), _cs(320))                 # c
            nc.gpsimd.tensor_tensor(out=_ko(32), in0=_cs(448),
                                    in1=_cs(320),
                                    op=mybir.AluOpType.subtract)    # s - c
            nc.gpsimd.tensor_tensor(out=_ko(64), in0=_cs(320),
                                    in1=_cs(448),
                                    op=mybir.AluOpType.add)         # c + s
            cv = coef[:]

            # remaining subtile loads, all upfront: store waits can never
            # delay a load (single SP queue, loads issued first); subtiles
            # 1-2 split in half so the still-filling pipeline starves less
            for st in range(1, 2 * NBLK):
                xts.append(xin.tile([128, N_STATE], F32R, tag="xin",
                                    name=f"xs{st}"))
            # subtiles 1-2 split in half so the filling pipeline
            # starves less
            for st in (1, 2):
                nc.sync.dma_start(out=xts[st][:, 0:512],
                                  in_=x_in[st * 128:(st + 1) * 128, 0:512])
                nc.sync.dma_start(out=xts[st][:, 512:1024],
                                  in_=x_in[st * 128:(st + 1) * 128, 512:1024])
            for st in range(3, 2 * NBLK):
                nc.sync.dma_start(out=xts[st][:],
                                  in_=x_in[st * 128:(st + 1) * 128, :])

            for blk in range(NBLK):
                for j in range(2):
                    x_t = xts[2 * blk + j]
                    c = (2 * blk + j) % CBLK
                    xT = xtp.tile([128, NCH, 128], F32R)
                    t_sb = tsb.tile([128, NCH, 192], F32)
                    tb = t_sb[:]
                    o_t = outp.tile([128, N_STATE], F32)
                    ob = o_t[:]
                    def transpose_group(g):
                        # transpose 4 chunks: feats -> partitions
                        tp = tpsum.tile([128, 4, 128], F32R, tag="tp")
                        for q in range(4):
                            ch = 4 * g + q
                            nc.tensor.transpose(
                                tp[:, q, :],
                                x_t[:, ch * 128:(ch + 1) * 128],
                                consts[:, 192:320],
                            )
                        nc.scalar.copy(out=xT[:, 4 * g:4 * (g + 1), :],
                                       in_=tp[:])

                    def transpose_chunks(g0, gn):
                        # per-mul-group staging: shortest startup chain
                        tp = tpsum.tile([128, 4, 128], F32R, tag="tp")
                        for a in range(gn):
                            ch = g0 + a
                            nc.tensor.transpose(
                                tp[:, a, :],
                                x_t[:, ch * 128:(ch + 1) * 128],
                                consts[:, 192:320],
                            )
                        nc.scalar.copy(out=xT[:, g0:g0 + gn, :],
                                       in_=tp[:, 0:gn, :])

                    first = (blk == 0 and j == 0)
                    if not first:
                        transpose_group(0)
                    adds_done = 0
                    groups = ((0, 2), (2, 2), (4, 2), (6, 2)) if first else \
                        ((0, 3), (3, 3), (6, 2))
                    for g0, gn in groups:
                        if first:
                            transpose_chunks(g0, gn)
                        elif g0 == 3:
                            # second transpose group after the first matmul
                            # group: PE unblocks DVE's group-0 mul earliest
                            transpose_group(1)
                        # [P|Q] per chunk; one matmul per PSUM bank
                        pq = ypsum.tile([128, 3, 512], F32)
                        for a in range(gn):
                            nc.tensor.matmul(
                                pq[:, a, 0:256], xT[:, g0 + a, :], consts[:, 0:256],
                                start=True, stop=True,
                            )
                        pqb = pq[:]
                        # k = [z1*c | z2*(s-c) | z3*(c+s)] per head (96 each)
                        nc.vector.tensor_mul(
                            bass.AP(tensor=tb.tensor,
                                    offset=tb.offset + g0 * 192,
                                    ap=[list(tb.ap[0]), [192, gn], [96, 2],
                                        [1, 96]]),
                            bass.AP(tensor=pqb.tensor, offset=pqb.offset,
                                    ap=[list(pqb.ap[0]), [512, gn], [96, 2],
                                        [1, 96]]),
                            bass.AP(tensor=cv.tensor,
                                    offset=cv.offset + c * 96,
                                    ap=[list(cv.ap[0]), [0, gn], [0, 2],
                                        [1, 96]]),
                        )
                        # ready chunks -> add + store; half-subtile granules
                        # normally, quarters on the final subtile so the tail
                        # after the very last mul is as short as possible
                        final = (blk == NBLK - 1 and j == 1)
                        gran = 4
                        while (g0 + gn) >= gran * (adds_done + 1):
                            h = adds_done
                            w = gran * 128
                            tb0 = tb.offset + h * gran * 192
                            # out_lo = k1 - k3 ; out_hi = k1 + k2; one of the
                            # four add-ops per subtile runs on DVE (Pool would
                            # otherwise pace the run), and the final subtile's
                            # trailing pair skips Pool's backlog entirely
                            for oi, (dcol, din, op) in enumerate(
                                    ((0, 64, mybir.AluOpType.subtract),
                                     (32, 32, mybir.AluOpType.add))):
                                dve = (final and h == 1) or \
                                    (h == 0 and oi == 0)
                                eng = nc.vector if dve else nc.gpsimd
                                eng.tensor_tensor(
                                    out=bass.AP(tensor=ob.tensor,
                                                offset=ob.offset + h * w + dcol,
                                                ap=[list(ob.ap[0]),
                                                    [128, gran], [64, 2],
                                                    [1, HALF]]),
                                    in0=bass.AP(tensor=tb.tensor, offset=tb0,
                                                ap=[list(tb.ap[0]),
                                                    [192, gran], [96, 2],
                                                    [1, HALF]]),
                                    in1=bass.AP(tensor=tb.tensor,
                                                offset=tb0 + din,
                                                ap=[list(tb.ap[0]),
                                                    [192, gran], [96, 2],
                                                    [1, HALF]]),
                                    op=op,
                                )
                            r0 = blk * 256 + j * 128
                            if h == 1:
                                # DMA-bound regime: one full-subtile store
                                # packs the (now binding) DMA stream best
                                nc.sync.dma_start(out=out_d[r0:r0 + 128, :],
                                                  in_=o_t[:])
                            adds_done += 1
    nc.compile()  # bacc: splits multi-sem waits into EventSemaphore insts
    return nc


def _compose_r(thetas, rotation_pairs, theta_scale, rotation_matrix):
    """Replicates reference._compose_rotation."""
    idx = rotation_pairs.astype(np.int32)
    th = thetas.astype(np.float32) * np.float32(theta_scale[0])
    R = np.eye(D, dtype=np.float32)
    for k in range(th.shape[0]):
        i, j = int(idx[k, 0]), int(idx[k, 1])
        ck, sk = np.float32(np.cos(th[k])), np.float32(np.sin(th[k]))
        G = np.eye(D, dtype=np.float32)
        G[i, i] = ck
        G[i, j] = -sk
        G[j, i] = sk
        G[j, j] = ck
        R = (R @ G).astype(np.float32)
    return (R @ rotation_matrix.astype(np.float32)).astype(np.float32)


def _build_rr(R):
    """[128, 256]: per head h (rows 64h:64h+64, cols 96h:96h+96) the block
    [R_even+R_odd | R_even | R_odd] -> z1 = u+v, z2 = u, z3 = v. Columns
    192:256 are zero padding so the f32r matmul keeps its 256-wide (1
    cycle/row) moving dimension."""
    w = np.concatenate([R[:, 0::2] + R[:, 1::2], R[:, 0::2], R[:, 1::2]],
                       axis=1).astype(np.float32)
    rr = np.zeros((128, 192), dtype=np.float32)
    rr[0:D, 0:96] = w
    rr[D:128, 96:192] = w
    return np.ascontiguousarray(rr)


def make_in_maps(x, thetas, rotation_pairs, theta_scale, rotation_matrix,
                 inv_freq):
    x = np.asarray(x, dtype=np.float32)
    R = _compose_r(
        np.asarray(thetas, np.float32),
        np.asarray(rotation_pairs, np.float32),
        np.asarray(theta_scale, np.float32),
        np.asarray(rotation_matrix, np.float32),
    )
    rr = _build_rr(R)
    invf = np.asarray(inv_freq, np.float32)
    pos = np.arange(S, dtype=np.float32)
    sinusoid = pos[:, None] * invf[None, :]               # [S, 32]
    cosf = np.cos(sinusoid).astype(np.float32)
    sinf = np.sin(sinusoid).astype(np.float32)

    in_maps = []
    for k in range(N_CORES):
        blk = slice(k * S_SH, (k + 1) * S_SH)
        # device builds [cos | sin-cos | cos+sin] rows on-chip; with
        # k1 = (u+v)cos, k2 = u(sin-cos), k3 = v(cos+sin):
        # out_lo = k1-k3 = u cos - v sin, out_hi = k1+k2 = u sin + v cos
        cc = cosf[blk].reshape(CBLK, 128, HALF)
        ss = sinf[blk].reshape(CBLK, 128, HALF)
        xs = np.ascontiguousarray(x[:, blk, :]).reshape(ROWS, N_STATE)
        consts = np.concatenate(
            [rr, np.eye(128, dtype=np.float32),
             cc.transpose(1, 0, 2).reshape(128, CBLK * HALF),
             ss.transpose(1, 0, 2).reshape(128, CBLK * HALF)], axis=1)
        in_maps.append({"x": xs, "consts": np.ascontiguousarray(consts)})
    return in_maps


def kernel(x, thetas, rotation_pairs, theta_scale, rotation_matrix, inv_freq):
    in_maps = make_in_maps(x, thetas, rotation_pairs, theta_scale,
                           rotation_matrix, inv_freq)
    if "nc" not in _compiled:
        _compiled["nc"] = _build_nc()
    res = run_bass_kernel_spmd(_compiled["nc"], in_maps,
                               list(range(N_CORES))).results

    out = np.empty((B, S, N_STATE), dtype=np.float32)
    for k in range(N_CORES):
        blk = slice(k * S_SH, (k + 1) * S_SH)
        out[:, blk, :] = res[k]["out"].reshape(B, S_SH, N_STATE)
    return out


# revision 68
# speedup vs baseline: 1.0036x; 1.0036x over previous
"""CombinedRotaryEmbedding Trainium2 kernel.

Math (per 64-dim head, per position s, with R = composed Givens @ rotation_matrix):
    u = x @ R[:, 0::2],  v = x @ R[:, 1::2]
    out = [u*cos - v*sin | u*sin + v*cos]     cos/sin = f(position, freq[32])

Karatsuba-style 3-product form: per head the matmul produces
    z1 = x @ (R_even+R_odd),  z2 = x @ R_even,  z3 = x @ R_odd
and the vector engine computes k = [z1*cos | z2*(sin-cos) | z3*(cos+sin)],
then out_lo = k1 - k3 and out_hi = k1 + k2: 1.5 multiplies per output
element (vs 2 for the direct form), keeping the DVE stream under the DMA
floor. The [z1|z2|z3] blocks for 2 heads (192 wide) extend to a 256-wide
float32r matmul (1 cycle/row); the 64 "pad" columns physically overlap the
identity block in SBUF, since their PSUM output is never read.

Kernel strategy (8-way data parallel over the sequence dim):
  - host: one [128, 576] constant block: [RR(192) | identity(128) |
    cos rows(128) | sin rows(128)]; gpsimd builds the [c|s-c|c+s]
    coefficient tile on-chip.
  - device, per core (x shard [2048 rows, 1024] = 16 subtiles of 128 rows):
      SP   : all 16 subtile loads upfront (subtiles 0-2 split for pipeline
             fill), then one full-subtile store each; loads first means a
             store's sem wait can never delay a load
      PE   : transpose x chunks via f32r identity, one 256-wide f32r matmul
             per 2-head chunk -> [z1|z2|z3] blocks in PSUM
      ACT  : copy transposed chunks PSUM -> SBUF (4-chunk groups)
      DVE  : k = [z1|z2|z3] * coeffs (PSUM -> SBUF, [3,3,2]-chunk groups)
             plus one of the four add-ops per subtile
      GPSIMD: out_lo/out_hi adds per half-subtile
    The DMA engine is the binding resource and runs with zero idle: total =
    DMA start (1966) + 47.4 us busy + store sem (900) + drain (744).
    Compute finishes ~3 us before the last store.
"""

import numpy as np

import concourse.bacc as bacc
import concourse.bass as bass
import concourse.tile as tile
from concourse import mybir
from concourse.bass_utils import run_bass_kernel_spmd

N_CORES = 8
B, S, N_STATE = 4, 4096, 1024
H, D = 16, 64            # heads, head dim
HALF = D // 2            # 32 rotary freqs
S_SH = S // N_CORES      # 512 positions per core
ROWS = B * S_SH          # 2048 rows of [1024] per core
NBLK = ROWS // 256       # 8 DMA blocks of 256 rows
CBLK = S_SH // 128       # 4 distinct position blocks per core
NCH = N_STATE // 128     # 8 two-head chunks per row
F32 = mybir.dt.float32
F32R = mybir.dt.float32r
BF16 = mybir.dt.bfloat16

_compiled = {}


def _build_nc():
    nc = bacc.Bacc("TRN2")
    x_in = nc.dram_tensor("x", [ROWS, N_STATE], F32R, kind="ExternalInput")
    # combined constant block [RR(192) | identity(128) | cos | sin rows]:
    # RR holds per-head [R_even+R_odd | R_even | R_odd] blocks; its 256-wide
    # f32r matmul view extends 64 cols into the identity (that PSUM output
    # range is never read)
    consts_in = nc.dram_tensor("consts", [128, 448], F32R,
                               kind="ExternalInput")
    out_d = nc.dram_tensor("out", [ROWS, N_STATE], F32, kind="ExternalOutput")

    with tile.TileContext(nc) as tc:
        with (
            tc.tile_pool(name="const", bufs=1) as const,
            tc.tile_pool(name="xin", bufs=2 * NBLK) as xin,
            tc.tile_pool(name="xtp", bufs=3) as xtp,
            tc.tile_pool(name="tpsum", bufs=2, space="PSUM") as tpsum,
            tc.tile_pool(name="ypsum", bufs=2, space="PSUM") as ypsum,
            tc.tile_pool(name="tsb", bufs=3) as tsb,
            # one buffer per subtile: output buffers must never gate compute
            # (stores drain late because loads occupy the DMA engines first)
            tc.tile_pool(name="outp", bufs=2 * NBLK) as outp,
        ):
            # first subtile load goes ahead of everything else, split in two
            # so the transpose->copy->matmul->mul chain starts as early as
            # possible (chunks 0-3 usable after a 256 KB transfer)
            xts = []
            x_t = xin.tile([128, N_STATE], F32R, tag="xin", name="xs0")
            nc.sync.dma_start(out=x_t[:, 0:512], in_=x_in[0:128, 0:512])
            xts.append(x_t)

            consts = const.tile([128, 448], F32R)
            nc.sync.dma_start(out=consts[:], in_=consts_in[:])
            nc.sync.dma_start(out=x_t[:, 512:1024], in_=x_in[0:128, 512:1024])
            zsc = const.tile([128, 128], F32)
            nc.gpsimd.memset(zsc[:], 0.0)
            identr = const.tile([128, 128], F32R)
            nc.gpsimd.tensor_copy(identr[:], zsc[:])
            nc.gpsimd.affine_select(
                out=identr[:], in_=identr[:],
                compare_op=mybir.AluOpType.not_equal, fill=1.0, base=0,
                pattern=[[-1, 128]], channel_multiplier=1)
            # build [cos | sin-cos | cos+sin] per position block from the
            # compact [cos | sin] rows (gpsimd is idle this early); DMAing
            # the compact form saves 182 ns on the binding DMA stream
            coef = const.tile([128, CBLK, 96], F32)
            cvb = consts[:].bitcast(F32)
            kb = coef[:]

            def _cs(off):
                # c rows at 192 + cb*32, s rows at 320 + cb*32
                return bass.AP(tensor=cvb.tensor, offset=cvb.offset + off,
                               ap=[list(cvb.ap[0]), [32, CBLK], [1, 32]])

            def _ko(dout):
                return bass.AP(tensor=kb.tensor, offset=kb.offset + dout,
                               ap=[list(kb.ap[0]), [96, CBLK], [1, 32]])

            nc.gpsimd.tensor_copy(_ko(0), _cs(192))                 # c
            nc.gpsimd.tensor_tensor(out=_ko(32), in0=_cs(320),
                                    in1=_cs(192),
                                    op=mybir.AluOpType.subtract)    # s - c
            nc.gpsimd.tensor_tensor(out=_ko(64), in0=_cs(192),
                                    in1=_cs(320),
                                    op=mybir.AluOpType.add)         # c + s
            cv = coef[:]

            # remaining subtile loads, all upfront: store waits can never
            # delay a load (single SP queue, loads issued first); subtiles
            # 1-2 split in half so the still-filling pipeline starves less
            for st in range(1, 2 * NBLK):
                xts.append(xin.tile([128, N_STATE], F32R, tag="xin",
                                    name=f"xs{st}"))
            # subtiles 1-2 split in half so the filling pipeline
            # starves less
            for st in (1, 2):
                nc.sync.dma_start(out=xts[st][:, 0:512],
                                  in_=x_in[st * 128:(st + 1) * 128, 0:512])
                nc.sync.dma_start(out=xts[st][:, 512:1024],
                                  in_=x_in[st * 128:(st + 1) * 128, 512:1024])
            for st in range(3, 2 * NBLK):
                nc.sync.dma_start(out=xts[st][:],
                                  in_=x_in[st * 128:(st + 1) * 128, :])

            for blk in range(NBLK):
                for j in range(2):
                    x_t = xts[2 * blk + j]
                    c = (2 * blk + j) % CBLK
                    xT = xtp.tile([128, NCH, 128], F32R)
                    t_sb = tsb.tile([128, NCH, 192], F32)
                    tb = t_sb[:]
                    o_t = outp.tile([128, N_STATE], F32)
                    ob = o_t[:]
                    def transpose_group(g):
                        # transpose 4 chunks: feats -> partitions
                        tp = tpsum.tile([128, 4, 128], F32R, tag="tp")
                        for q in range(4):
                            ch = 4 * g + q
                            nc.tensor.transpose(
                                tp[:, q, :],
                                x_t[:, ch * 128:(ch + 1) * 128],
                                identr[:],
                            )
                        nc.scalar.copy(out=xT[:, 4 * g:4 * (g + 1), :],
                                       in_=tp[:])

                    def transpose_chunks(g0, gn):
                        # per-mul-group staging: shortest startup chain
                        tp = tpsum.tile([128, 4, 128], F32R, tag="tp")
                        for a in range(gn):
                            ch = g0 + a
                            nc.tensor.transpose(
                                tp[:, a, :],
                                x_t[:, ch * 128:(ch + 1) * 128],
                                identr[:],
                            )
                        nc.scalar.copy(out=xT[:, g0:g0 + gn, :],
                                       in_=tp[:, 0:gn, :])

                    first = (blk == 0 and j == 0)
                    if not first:
                        transpose_group(0)
                    adds_done = 0
                    groups = ((0, 2), (2, 2), (4, 2), (6, 2)) if first else \
                        ((0, 3), (3, 3), (6, 2))
                    for g0, gn in groups:
                        if first:
                            transpose_chunks(g0, gn)
                        elif g0 == 3:
                            # second transpose group after the first matmul
                            # group: PE unblocks DVE's group-0 mul earliest
                            transpose_group(1)
                        # [P|Q] per chunk; one matmul per PSUM bank
                        pq = ypsum.tile([128, 3, 512], F32)
                        for a in range(gn):
                            nc.tensor.matmul(
                                pq[:, a, 0:256], xT[:, g0 + a, :], consts[:, 0:256],
                                start=True, stop=True,
                            )
                        pqb = pq[:]
                        # k = [z1*c | z2*(s-c) | z3*(c+s)] per head (96 each)
                        nc.vector.tensor_mul(
                            bass.AP(tensor=tb.tensor,
                                    offset=tb.offset + g0 * 192,
                                    ap=[list(tb.ap[0]), [192, gn], [96, 2],
                                        [1, 96]]),
                            bass.AP(tensor=pqb.tensor, offset=pqb.offset,
                                    ap=[list(pqb.ap[0]), [512, gn], [96, 2],
                                        [1, 96]]),
                            bass.AP(tensor=cv.tensor,
                                    offset=cv.offset + c * 96,
                                    ap=[list(cv.ap[0]), [0, gn], [0, 2],
                                        [1, 96]]),
                        )
                        # ready chunks -> add + store; half-subtile granules
                        # normally, quarters on the final subtile so the tail
                        # after the very last mul is as short as possible
                        final = (blk == NBLK - 1 and j == 1)
                        gran = 4
                        while (g0 + gn) >= gran * (adds_done + 1):
                            h = adds_done
                            w = gran * 128
                            tb0 = tb.offset + h * gran * 192
                            # out_lo = k1 - k3 ; out_hi = k1 + k2; one of the
                            # four add-ops per subtile runs on DVE (Pool would
                            # otherwise pace the run), and the final subtile's
                            # trailing pair skips Pool's backlog entirely
                            for oi, (dcol, din, op) in enumerate(
                                    ((0, 64, mybir.AluOpType.subtract),
                                     (32, 32, mybir.AluOpType.add))):
                                dve = (final and h == 1) or \
                                    (h == 0 and oi == 0)
                                eng = nc.vector if dve else nc.gpsimd
                                eng.tensor_tensor(
                                    out=bass.AP(tensor=ob.tensor,
                                                offset=ob.offset + h * w + dcol,
                                                ap=[list(ob.ap[0]),
                                                    [128, gran], [64, 2],
                                                    [1, HALF]]),
                                    in0=bass.AP(tensor=tb.tensor, offset=tb0,
                                                ap=[list(tb.ap[0]),
                                                    [192, gran], [96, 2],
                                                    [1, HALF]]),
                                    in1=bass.AP(tensor=tb.tensor,
                                                offset=tb0 + din,
                                                ap=[list(tb.ap[0]),
                                                    [192, gran], [96, 2],
                                                    [1, HALF]]),
                                    op=op,
                                )
                            r0 = blk * 256 + j * 128
                            if h == 1:
                                # DMA-bound regime: one full-subtile store
                                # packs the (now binding) DMA stream best
                                nc.sync.dma_start(out=out_d[r0:r0 + 128, :],
                                                  in_=o_t[:])
                            adds_done += 1
    nc.compile()  # bacc: splits multi-sem waits into EventSemaphore insts
    return nc


def _compose_r(thetas, rotation_pairs, theta_scale, rotation_matrix):
    """Replicates reference._compose_rotation."""
    idx = rotation_pairs.astype(np.int32)
    th = thetas.astype(np.float32) * np.float32(theta_scale[0])
    R = np.eye(D, dtype=np.float32)
    for k in range(th.shape[0]):
        i, j = int(idx[k, 0]), int(idx[k, 1])
        ck, sk = np.float32(np.cos(th[k])), np.float32(np.sin(th[k]))
        G = np.eye(D, dtype=np.float32)
        G[i, i] = ck
        G[i, j] = -sk
        G[j, i] = sk
        G[j, j] = ck
        R = (R @ G).astype(np.float32)
    return (R @ rotation_matrix.astype(np.float32)).astype(np.float32)


def _build_rr(R):
    """[128, 256]: per head h (rows 64h:64h+64, cols 96h:96h+96) the block
    [R_even+R_odd | R_even | R_odd] -> z1 = u+v, z2 = u, z3 = v. Columns
    192:256 are zero padding so the f32r matmul keeps its 256-wide (1
    cycle/row) moving dimension."""
    w = np.concatenate([R[:, 0::2] + R[:, 1::2], R[:, 0::2], R[:, 1::2]],
                       axis=1).astype(np.float32)
    rr = np.zeros((128, 192), dtype=np.float32)
    rr[0:D, 0:96] = w
    rr[D:128, 96:192] = w
    return np.ascontiguousarray(rr)


def make_in_maps(x, thetas, rotation_pairs, theta_scale, rotation_matrix,
                 inv_freq):
    x = np.asarray(x, dtype=np.float32)
    R = _compose_r(
        np.asarray(thetas, np.float32),
        np.asarray(rotation_pairs, np.float32),
        np.asarray(theta_scale, np.float32),
        np.asarray(rotation_matrix, np.float32),
    )
    rr = _build_rr(R)
    invf = np.asarray(inv_freq, np.float32)
    pos = np.arange(S, dtype=np.float32)
    sinusoid = pos[:, None] * invf[None, :]               # [S, 32]
    cosf = np.cos(sinusoid).astype(np.float32)
    sinf = np.sin(sinusoid).astype(np.float32)

    in_maps = []
    for k in range(N_CORES):
        blk = slice(k * S_SH, (k + 1) * S_SH)
        # device builds [cos | sin-cos | cos+sin] rows on-chip; with
        # k1 = (u+v)cos, k2 = u(sin-cos), k3 = v(cos+sin):
        # out_lo = k1-k3 = u cos - v sin, out_hi = k1+k2 = u sin + v cos
        cc = cosf[blk].reshape(CBLK, 128, HALF)
        ss = sinf[blk].reshape(CBLK, 128, HALF)
        xs = np.ascontiguousarray(x[:, blk, :]).reshape(ROWS, N_STATE)
        consts = np.concatenate(
            [rr, cc.transpose(1, 0, 2).reshape(128, CBLK * HALF),
             ss.transpose(1, 0, 2).reshape(128, CBLK * HALF)], axis=1)
        in_maps.append({"x": xs, "consts": np.ascontiguousarray(consts)})
    return in_maps


def kernel(x, thetas, rotation_pairs, theta_scale, rotation_matrix, inv_freq):
    in_maps = make_in_maps(x, thetas, rotation_pairs, theta_scale,
                           rotation_matrix, inv_freq)
    if "nc" not in _compiled:
        _compiled["nc"] = _build_nc()
    res = run_bass_kernel_spmd(_compiled["nc"], in_maps,
                               list(range(N_CORES))).results

    out = np.empty((B, S, N_STATE), dtype=np.float32)
    for k in range(N_CORES):
        blk = slice(k * S_SH, (k + 1) * S_SH)
        out[:, blk, :] = res[k]["out"].reshape(B, S_SH, N_STATE)
    return out
